# revision 5
# baseline (speedup 1.0000x reference)
"""TRN2 Bass kernel for nn_Attention_73839077752929.

Computes (matching the reference, which has the source bug k = v = q):
    q = x @ Wq^T + bq          (only the q-slice of Wqkv is ever used)
    a = softmax(causal(q q^T / 8)) @ q      per head
    y = a @ Wproj^T + bproj

Sharding: 8 cores = 4 batches x 2 head-groups (6 heads each).
Each core computes a partial projection output for its batch; the host
sums the two partials per batch and adds the projection bias.

On-core scheme (all matmuls fp32r; transposed-probability formulation):
    qT[d,t]   = wqT^T @ xT (+bias)                      [pairs of heads on 128 partitions]
    V_i       = PE-transpose of qT, with a ones column  [k,(h,d|1)]
    PT_i[k,q] = exp((S^T + maskT)/8)  where S^T block = qT_k^T @ qT_q
    OT'[d|1,q] = sum_i V_i^T @ PT_i    (extra row = softmax denominators)
    aT       *= bcast(1/denominators)   (deferred normalization)
    y[t,o]    = aT^T @ wpT
"""

import os

import numpy as np

N_CORES = 8
NB, NS, NF = 4, 2048, 768
N_HEADS_TOTAL = 12
HD = 64
NH = 6  # heads per core
DL = NH * HD  # 384 local dims
NPAIR = NH // 2  # 3 head pairs (128 partitions each)
NKB = NS // 128  # 16 k-blocks
NJC = NS // 512  # 4 q-chunks
NFC = NF // 128  # 6 feature chunks

_COMPILED = {}


def _build():
    import concourse.bacc as bacc
    import concourse.bass as bass
    import concourse.mybir as mybir
    import concourse.tile as tile
    from concourse.masks import make_identity

    F32 = mybir.dt.float32
    F32R = mybir.dt.float32r

    nc = bacc.Bacc(trn_type="TRN2", target_bir_lowering=False)

    xT_d = nc.dram_tensor("xT", [NF, NS], F32, kind="ExternalInput").ap()
    wqT_d = nc.dram_tensor("wqT", [NF, DL], F32, kind="ExternalInput").ap()
    bq_d = nc.dram_tensor("bq", [NPAIR, 128], F32, kind="ExternalInput").ap()
    wpT_d = nc.dram_tensor("wpT", [DL, NF], F32, kind="ExternalInput").ap()
    y_d = nc.dram_tensor("y", [NS, NF], F32, kind="ExternalOutput").ap()

    with tile.TileContext(nc) as tc:
        with (
            tc.tile_pool(name="const", bufs=1) as constp,
            tc.tile_pool(name="w", bufs=1) as wp,
            tc.tile_pool(name="big", bufs=1) as bigp,
            tc.tile_pool(name="pt", bufs=6) as ptp,
            tc.tile_pool(name="ys", bufs=2) as ysp,
            tc.tile_pool(name="ps_s", bufs=4, space="PSUM") as ps_s,
            tc.tile_pool(name="ps_o", bufs=2, space="PSUM") as ps_o,
            tc.tile_pool(name="ps_m", bufs=2, space="PSUM") as ps_m,
        ):
            # ---------------- constants ----------------
            identf = constp.tile([128, 128], F32, tag="identf")
            make_identity(nc, identf[:])
            ident = constp.tile([128, 128], F32R, tag="ident")
            maskT = constp.tile([128, 128], F32, tag="maskT")
            # transposed causal mask: keep 0 where k <= q, else -1e10
            nc.gpsimd.memset(maskT[:], 0.0)
            nc.gpsimd.affine_select(
                out=maskT[:],
                in_=maskT[:],
                compare_op=mybir.AluOpType.is_ge,
                fill=-1e10,
                base=0,
                pattern=[[1, 128]],
                channel_multiplier=-1,
            )
            zerof = constp.tile([128, 512], F32, tag="zerof")
            nc.gpsimd.memset(zerof[:], 0.0)
            onesf = constp.tile([128, 64], F32, tag="onesf")
            nc.gpsimd.memset(onesf[:], 1.0)
            # E2: [64,128] selector, rows 0 / 32 broadcast to head halves
            e2f = constp.tile([64, 128], F32, tag="e2f")
            nc.gpsimd.memset(e2f[:], 0.0)
            nc.gpsimd.memset(e2f[0:1, 0:64], 1.0)
            nc.gpsimd.memset(e2f[32:33, 64:128], 1.0)
            e2 = constp.tile([64, 128], F32R, tag="e2")
            zerosR = constp.tile([128, 384], F32R, tag="zerosR")
            onesR = constp.tile([128, NH], F32R, tag="onesR")
            with nc.allow_low_precision(reason="f32r constants"):
                nc.vector.tensor_copy(ident[:], identf[:])
                nc.vector.tensor_copy(e2[:], e2f[:])
                nc.vector.tensor_copy(zerosR[:], zerof[:, 0:384])
                nc.vector.tensor_copy(onesR[:], onesf[:, 0:NH])
            bq_t = constp.tile([128, NPAIR], F32, tag="bq")
            nc.sync.dma_start(bq_t[:], bq_d.rearrange("c p -> p c"))

            # ---------------- weights / activations ----------------
            wqT = wp.tile([128, NFC, DL], F32R, tag="wqT")
            nc.sync.dma_start(
                wqT[:], wqT_d.rearrange("(c p) d -> p c d", p=128).bitcast(F32R)
            )
            wpT = wp.tile([128, NPAIR, NF], F32R, tag="wpT")
            nc.sync.dma_start(
                wpT[:], wpT_d.rearrange("(c p) o -> p c o", p=128).bitcast(F32R)
            )
            xT = wp.tile([128, NFC, NS], F32R, tag="xT")
            xT_src = xT_d.rearrange("(c p) t -> p c t", p=128).bitcast(F32R)
            for tck in range(NS // 512):
                nc.scalar.dma_start(
                    xT[:, :, bass.ts(tck, 512)], xT_src[:, :, bass.ts(tck, 512)]
                )

            # ---------------- qT = wqT^T @ xT + bq ----------------
            qT = bigp.tile([128, NPAIR, NS], F32R, tag="qT")
            for tck in range(NS // 512):
                for pc in range(NPAIR):
                    pq = ps_m.tile([128, 512], F32, tag="m")
                    for fc in range(NFC):
                        nc.tensor.matmul(
                            pq[:],
                            lhsT=wqT[:, fc, pc * 128 : (pc + 1) * 128],
                            rhs=xT[:, fc, bass.ts(tck, 512)],
                            start=(fc == 0),
                            stop=(fc == NFC - 1),
                        )
                    with nc.allow_low_precision(reason="f32r rounding"):
                        nc.vector.tensor_scalar_add(
                            qT[:, pc, bass.ts(tck, 512)],
                            pq[:],
                            bq_t[:, pc : pc + 1],
                        )

            # ---------------- V_i via PE transpose ----------------
            vt = bigp.tile([128, NKB, NH, HD + 1], F32R, tag="vt")
            for i in range(NKB):
                for pc in range(NPAIR):
                    pv = ps_m.tile([128, 512], F32, tag="m")
                    with nc.allow_low_precision(reason="transpose is movement"):
                        nc.tensor.transpose(
                            pv[:, :128].bitcast(F32R),
                            qT[:, pc, bass.ts(i, 128)],
                            ident[:],
                        )
                    with nc.allow_low_precision(reason="f32r rounding"):
                        nc.vector.tensor_copy(
                            vt[:, i, 2 * pc : 2 * pc + 2, 0:HD],
                            pv[:, :128].rearrange("k (h d) -> k h d", h=2),
                        )
                with nc.allow_low_precision(reason="f32r constants"):
                    nc.vector.tensor_copy(
                        vt[:, i, :, HD : HD + 1],
                        onesR[:].rearrange("p (h u) -> p h u", u=1),
                    )

            # ---------------- attention ----------------
            aT = bigp.tile([128, NPAIR, NS], F32R, tag="aT")
            rs_tiles = []
            for pc in range(NPAIR):
                rs_pc = bigp.tile([64, NS], F32R, tag=f"rs{pc}")
                rs_tiles.append(rs_pc)
                # zero once: garbage rows would poison the bcast matmul
                for jc in range(NJC):
                    with nc.allow_low_precision(reason="f32r zeros"):
                        nc.vector.tensor_copy(
                            rs_pc[:, bass.ts(jc, 512)], zerof[0:64, :]
                        )

            for jc in range(NJC):
                nkb = 4 * jc + 4
                for h in range(NH):
                    pc, h2 = h // 2, h % 2
                    q_lo, q_hi = h2 * HD, (h2 + 1) * HD
                    po = ps_o.tile([HD + 1, 512], F32, tag="o")
                    for i in range(nkb):
                        off = max(0, (i - 4 * jc) * 128)
                        pt_i = ptp.tile([128, 512], F32R, tag="pt")
                        psT = ps_s.tile([128, 512], F32, tag="s")
                        nc.tensor.matmul(
                            psT[:, off:512],
                            lhsT=qT[q_lo:q_hi, pc, bass.ts(i, 128)],
                            rhs=qT[q_lo:q_hi, pc, jc * 512 + off : (jc + 1) * 512],
                            start=True,
                            stop=True,
                        )
                        if i >= 4 * jc:
                            nc.vector.tensor_tensor(
                                psT[:, off : off + 128],
                                psT[:, off : off + 128],
                                maskT[:],
                                mybir.AluOpType.add,
                            )
                        with nc.allow_low_precision(reason="f32r rounding"):
                            nc.scalar.activation(
                                pt_i[:, off:512],
                                psT[:, off:512],
                                mybir.ActivationFunctionType.Exp,
                                scale=0.125,
                            )
                        nc.tensor.matmul(
                            po[:, off:512],
                            lhsT=vt[:, i, h, :],
                            rhs=pt_i[:, off:512],
                            start=(i == 0),
                            stop=(i == nkb - 1),
                            skip_group_check=True,
                        )
                    with nc.allow_low_precision(reason="f32r rounding"):
                        nc.vector.tensor_copy(
                            aT[q_lo:q_hi, pc, bass.ts(jc, 512)],
                            po[0:HD, :],
                        )
                        nc.vector.reciprocal(
                            rs_tiles[pc][32 * h2 : 32 * h2 + 1, bass.ts(jc, 512)],
                            po[HD : HD + 1, :],
                        )

                # normalize chunk jc, then project its 4 token blocks
                for pc in range(NPAIR):
                    prb = ps_m.tile([128, 512], F32, tag="m")
                    nc.tensor.matmul(
                        prb[:],
                        lhsT=e2[:],
                        rhs=rs_tiles[pc][:, bass.ts(jc, 512)],
                        start=True,
                        stop=True,
                    )
                    with nc.allow_low_precision(reason="f32r rounding"):
                        nc.vector.tensor_tensor(
                            aT[:, pc, bass.ts(jc, 512)],
                            aT[:, pc, bass.ts(jc, 512)],
                            prb[:],
                            mybir.AluOpType.mult,
                        )
                for tb in range(4 * jc, 4 * jc + 4):
                    ysb = ysp.tile([128, NF], F32, tag="y")
                    for o0, on in ((0, 512), (512, 256)):
                        py = ps_m.tile([128, 512], F32, tag="m")
                        for pc in range(NPAIR):
                            nc.tensor.matmul(
                                py[:, 0:on],
                                lhsT=aT[:, pc, bass.ts(tb, 128)],
                                rhs=wpT[:, pc, o0 : o0 + on],
                                start=(pc == 0),
                                stop=(pc == NPAIR - 1),
                            )
                        nc.vector.tensor_copy(ysb[:, o0 : o0 + on], py[:, 0:on])
                    nc.gpsimd.dma_start(y_d[bass.ts(tb, 128), :], ysb[:])

    nc.compile()
    return nc


def kernel(x, Wqkv_w, Wqkv_b, Wproj_w, Wproj_b):
    from concourse.bass_utils import run_bass_kernel_spmd

    x = np.asarray(x, dtype=np.float32)
    Wqkv_w = np.asarray(Wqkv_w, dtype=np.float32)
    Wqkv_b = np.asarray(Wqkv_b, dtype=np.float32)
    Wproj_w = np.asarray(Wproj_w, dtype=np.float32)
    Wproj_b = np.asarray(Wproj_b, dtype=np.float32)

    if "nc" not in _COMPILED:
        _COMPILED["nc"] = _build()
    nc = _COMPILED["nc"]

    in_maps = []
    for c in range(N_CORES):
        b, g = c // 2, c % 2
        sl = slice(g * DL, (g + 1) * DL)
        in_maps.append(
            {
                "xT": np.ascontiguousarray(x[b].T),
                "wqT": np.ascontiguousarray(Wqkv_w[:NF][sl].T),
                "bq": np.ascontiguousarray(Wqkv_b[:NF][sl].reshape(NPAIR, 128)),
                "wpT": np.ascontiguousarray(Wproj_w[:, sl].T),
            }
        )

    trace = bool(int(os.environ.get("KERNEL_TRACE", "0")))
    res = run_bass_kernel_spmd(
        nc,
        in_maps,
        list(range(N_CORES)),
        trace=trace,
        trace_cores=list(range(N_CORES)) if trace else None,
    )
    if trace:
        _COMPILED["exec_time_ns"] = res.exec_time_ns
        _COMPILED["mean_exec_time_ns"] = res.mean_exec_time_ns
        _COMPILED["results_obj"] = res

    y = np.empty((NB, NS, NF), dtype=np.float32)
    for b in range(NB):
        y[b] = res.results[2 * b]["y"] + res.results[2 * b + 1]["y"] + Wproj_b
    return y


# revision 10
# speedup vs baseline: 1.0132x; 1.0132x over previous
"""TRN2 Bass kernel for nn_Attention_73839077752929.

Computes (matching the reference, which has the source bug k = v = q):
    q = x @ Wq^T + bq          (only the q-slice of Wqkv is ever used)
    a = softmax(causal(q q^T / 8)) @ q      per head
    y = a @ Wproj^T + bproj

Sharding: 8 cores = 4 batches x 2 head-groups (6 heads each).
Each core computes a partial projection output for its batch; the host
sums the two partials per batch and adds the projection bias.

On-core scheme (all matmuls fp32r; transposed-probability formulation):
    qT[d,t]   = wqT^T @ xT (+bias)                      [pairs of heads on 128 partitions]
    V_i       = PE-transpose of qT, with a ones column  [k,(h,d|1)]
    PT_i[k,q] = exp((S^T + maskT)/8)  where S^T block = qT_k^T @ qT_q
    OT'[d|1,q] = sum_i V_i^T @ PT_i    (extra row = softmax denominators)
    aT       *= bcast(1/denominators)   (deferred normalization)
    y[t,o]    = aT^T @ wpT
"""

import os

import numpy as np

N_CORES = 8
NB, NS, NF = 4, 2048, 768
N_HEADS_TOTAL = 12
HD = 64
NH = 6  # heads per core
DL = NH * HD  # 384 local dims
NPAIR = NH // 2  # 3 head pairs (128 partitions each)
NKB = NS // 128  # 16 k-blocks
NJC = NS // 512  # 4 q-chunks
NFC = NF // 128  # 6 feature chunks

_COMPILED = {}


def _build():
    import concourse.bacc as bacc
    import concourse.bass as bass
    import concourse.mybir as mybir
    import concourse.tile as tile
    from concourse.masks import make_identity

    F32 = mybir.dt.float32
    F32R = mybir.dt.float32r

    nc = bacc.Bacc(trn_type="TRN2", target_bir_lowering=False)

    xT_d = nc.dram_tensor("xT", [NF, NS], F32, kind="ExternalInput").ap()
    wqT_d = nc.dram_tensor("wqT", [NF, DL], F32, kind="ExternalInput").ap()
    bq_d = nc.dram_tensor("bq", [NPAIR, 128], F32, kind="ExternalInput").ap()
    wpT_d = nc.dram_tensor("wpT", [DL, NF], F32, kind="ExternalInput").ap()
    y_d = nc.dram_tensor("y", [NS, NF], F32, kind="ExternalOutput").ap()

    with tile.TileContext(nc) as tc:
        with (
            tc.tile_pool(name="const", bufs=1) as constp,
            tc.tile_pool(name="w", bufs=1) as wp,
            tc.tile_pool(name="big", bufs=1) as bigp,
            tc.tile_pool(name="pt", bufs=10) as ptp,
            tc.tile_pool(name="ys", bufs=2) as ysp,
            tc.tile_pool(name="ps_s", bufs=4, space="PSUM") as ps_s,
            tc.tile_pool(name="ps_o", bufs=2, space="PSUM") as ps_o,
            tc.tile_pool(name="ps_m", bufs=2, space="PSUM") as ps_m,
        ):
            # ---------------- constants ----------------
            identf = constp.tile([128, 128], F32, tag="identf")
            make_identity(nc, identf[:])
            ident = constp.tile([128, 128], F32R, tag="ident")
            maskMf = constp.tile([128, 128], F32, tag="maskMf")
            # multiplicative transposed-causal mask: 1.0 where k <= q else 0.0
            nc.gpsimd.memset(maskMf[:], 1.0)
            nc.gpsimd.affine_select(
                out=maskMf[:],
                in_=maskMf[:],
                compare_op=mybir.AluOpType.is_ge,
                fill=0.0,
                base=0,
                pattern=[[1, 128]],
                channel_multiplier=-1,
            )
            maskM = constp.tile([128, 128], F32R, tag="maskM")
            zerof = constp.tile([128, 512], F32, tag="zerof")
            nc.gpsimd.memset(zerof[:], 0.0)
            onesf = constp.tile([128, 64], F32, tag="onesf")
            nc.gpsimd.memset(onesf[:], 1.0)
            # E2: [64,128] selector, rows 0 / 32 broadcast to head halves
            e2f = constp.tile([64, 128], F32, tag="e2f")
            nc.gpsimd.memset(e2f[:], 0.0)
            nc.gpsimd.memset(e2f[0:1, 0:64], 1.0)
            nc.gpsimd.memset(e2f[32:33, 64:128], 1.0)
            e2 = constp.tile([64, 128], F32R, tag="e2")
            zerosR = constp.tile([128, 384], F32R, tag="zerosR")
            onesR = constp.tile([128, NH], F32R, tag="onesR")
            with nc.allow_low_precision(reason="f32r constants"):
                nc.vector.tensor_copy(maskM[:], maskMf[:])
                nc.vector.tensor_copy(ident[:], identf[:])
                nc.vector.tensor_copy(e2[:], e2f[:])
                nc.vector.tensor_copy(zerosR[:], zerof[:, 0:384])
                nc.vector.tensor_copy(onesR[:], onesf[:, 0:NH])
            bq_t = constp.tile([128, NPAIR], F32, tag="bq")
            nc.sync.dma_start(bq_t[:], bq_d.rearrange("c p -> p c"))

            # ---------------- weights / activations ----------------
            wqT = wp.tile([128, NFC, DL], F32R, tag="wqT")
            nc.sync.dma_start(
                wqT[:], wqT_d.rearrange("(c p) d -> p c d", p=128).bitcast(F32R)
            )
            wpT = wp.tile([128, NPAIR, NF], F32R, tag="wpT")
            nc.sync.dma_start(
                wpT[:], wpT_d.rearrange("(c p) o -> p c o", p=128).bitcast(F32R)
            )
            xT = wp.tile([128, NFC, NS], F32R, tag="xT")
            xT_src = xT_d.rearrange("(c p) t -> p c t", p=128).bitcast(F32R)
            for tck in range(NS // 512):
                nc.scalar.dma_start(
                    xT[:, :, bass.ts(tck, 512)], xT_src[:, :, bass.ts(tck, 512)]
                )

            # ---------------- qT = wqT^T @ xT + bq ----------------
            qT = bigp.tile([128, NPAIR, NS], F32R, tag="qT")
            for tck in range(NS // 512):
                for pc in range(NPAIR):
                    pq = ps_m.tile([128, 512], F32, tag="m")
                    for fc in range(NFC):
                        nc.tensor.matmul(
                            pq[:],
                            lhsT=wqT[:, fc, pc * 128 : (pc + 1) * 128],
                            rhs=xT[:, fc, bass.ts(tck, 512)],
                            start=(fc == 0),
                            stop=(fc == NFC - 1),
                        )
                    with nc.allow_low_precision(reason="f32r rounding"):
                        nc.vector.tensor_scalar_add(
                            qT[:, pc, bass.ts(tck, 512)],
                            pq[:],
                            bq_t[:, pc : pc + 1],
                        )

            # ---------------- V_i via PE transpose ----------------
            vt = bigp.tile([128, NKB, NH, HD + 1], F32R, tag="vt")
            for i in range(NKB):
                for pc in range(NPAIR):
                    pv = ps_m.tile([128, 512], F32, tag="m")
                    with nc.allow_low_precision(reason="transpose is movement"):
                        nc.tensor.transpose(
                            pv[:, :128].bitcast(F32R),
                            qT[:, pc, bass.ts(i, 128)],
                            ident[:],
                        )
                    with nc.allow_low_precision(reason="f32r rounding"):
                        nc.vector.tensor_copy(
                            vt[:, i, 2 * pc : 2 * pc + 2, 0:HD],
                            pv[:, :128].rearrange("k (h d) -> k h d", h=2),
                        )
                with nc.allow_low_precision(reason="f32r constants"):
                    nc.vector.tensor_copy(
                        vt[:, i, :, HD : HD + 1],
                        onesR[:].rearrange("p (h u) -> p h u", u=1),
                    )

            # ---------------- attention ----------------
            aT = bigp.tile([128, NPAIR, NS], F32R, tag="aT")
            rs_tiles = []
            for pc in range(NPAIR):
                rs_pc = bigp.tile([64, NS], F32R, tag=f"rs{pc}")
                rs_tiles.append(rs_pc)
                # zero once: garbage rows would poison the bcast matmul
                for jc in range(NJC):
                    with nc.allow_low_precision(reason="f32r zeros"):
                        nc.vector.tensor_copy(
                            rs_pc[:, bass.ts(jc, 512)], zerof[0:64, :]
                        )

            for jc in range(NJC):
                nkb = 4 * jc + 4
                for h in range(NH):
                    pc, h2 = h // 2, h % 2
                    q_lo, q_hi = h2 * HD, (h2 + 1) * HD
                    po = ps_o.tile([HD + 1, 512], F32, tag="o")
                    for i in range(nkb):
                        off = max(0, (i - 4 * jc) * 128)
                        pt_i = ptp.tile([128, 512], F32R, tag="pt")
                        psT = ps_s.tile([128, 512], F32, tag="s")
                        nc.tensor.matmul(
                            psT[:, off:512],
                            lhsT=qT[q_lo:q_hi, pc, bass.ts(i, 128)],
                            rhs=qT[q_lo:q_hi, pc, jc * 512 + off : (jc + 1) * 512],
                            start=True,
                            stop=True,
                        )
                        with nc.allow_low_precision(reason="f32r rounding"):
                            nc.scalar.activation(
                                pt_i[:, off:512],
                                psT[:, off:512],
                                mybir.ActivationFunctionType.Exp,
                                scale=0.125,
                            )
                        if i >= 4 * jc:
                            with nc.allow_low_precision(reason="f32r mask"):
                                nc.gpsimd.tensor_tensor(
                                    pt_i[:, off : off + 128],
                                    pt_i[:, off : off + 128],
                                    maskM[:],
                                    mybir.AluOpType.mult,
                                )
                        nc.tensor.matmul(
                            po[:, off:512],
                            lhsT=vt[:, i, h, :],
                            rhs=pt_i[:, off:512],
                            start=(i == 0),
                            stop=(i == nkb - 1),
                            skip_group_check=True,
                        )
                    with nc.allow_low_precision(reason="f32r rounding"):
                        nc.vector.tensor_copy(
                            aT[q_lo:q_hi, pc, bass.ts(jc, 512)],
                            po[0:HD, :],
                        )
                        nc.vector.reciprocal(
                            rs_tiles[pc][32 * h2 : 32 * h2 + 1, bass.ts(jc, 512)],
                            po[HD : HD + 1, :],
                        )

                # normalize chunk jc, then project its 4 token blocks
                for pc in range(NPAIR):
                    prb = ps_m.tile([128, 512], F32, tag="m")
                    nc.tensor.matmul(
                        prb[:],
                        lhsT=e2[:],
                        rhs=rs_tiles[pc][:, bass.ts(jc, 512)],
                        start=True,
                        stop=True,
                    )
                    with nc.allow_low_precision(reason="f32r rounding"):
                        nc.vector.tensor_tensor(
                            aT[:, pc, bass.ts(jc, 512)],
                            aT[:, pc, bass.ts(jc, 512)],
                            prb[:],
                            mybir.AluOpType.mult,
                        )
                for tb in range(4 * jc, 4 * jc + 4):
                    ysb = ysp.tile([128, NF], F32, tag="y")
                    for o0, on in ((0, 512), (512, 256)):
                        py = ps_m.tile([128, 512], F32, tag="m")
                        for pc in range(NPAIR):
                            nc.tensor.matmul(
                                py[:, 0:on],
                                lhsT=aT[:, pc, bass.ts(tb, 128)],
                                rhs=wpT[:, pc, o0 : o0 + on],
                                start=(pc == 0),
                                stop=(pc == NPAIR - 1),
                            )
                        nc.vector.tensor_copy(ysb[:, o0 : o0 + on], py[:, 0:on])
                    nc.gpsimd.dma_start(y_d[bass.ts(tb, 128), :], ysb[:])

    nc.compile()
    return nc


def kernel(x, Wqkv_w, Wqkv_b, Wproj_w, Wproj_b):
    from concourse.bass_utils import run_bass_kernel_spmd

    x = np.asarray(x, dtype=np.float32)
    Wqkv_w = np.asarray(Wqkv_w, dtype=np.float32)
    Wqkv_b = np.asarray(Wqkv_b, dtype=np.float32)
    Wproj_w = np.asarray(Wproj_w, dtype=np.float32)
    Wproj_b = np.asarray(Wproj_b, dtype=np.float32)

    if "nc" not in _COMPILED:
        _COMPILED["nc"] = _build()
    nc = _COMPILED["nc"]

    in_maps = []
    for c in range(N_CORES):
        b, g = c // 2, c % 2
        sl = slice(g * DL, (g + 1) * DL)
        in_maps.append(
            {
                "xT": np.ascontiguousarray(x[b].T),
                "wqT": np.ascontiguousarray(Wqkv_w[:NF][sl].T),
                "bq": np.ascontiguousarray(Wqkv_b[:NF][sl].reshape(NPAIR, 128)),
                "wpT": np.ascontiguousarray(Wproj_w[:, sl].T),
            }
        )

    trace = bool(int(os.environ.get("KERNEL_TRACE", "0")))
    res = run_bass_kernel_spmd(
        nc,
        in_maps,
        list(range(N_CORES)),
        trace=trace,
        trace_cores=list(range(N_CORES)) if trace else None,
    )
    if trace:
        _COMPILED["exec_time_ns"] = res.exec_time_ns
        _COMPILED["mean_exec_time_ns"] = res.mean_exec_time_ns
        _COMPILED["results_obj"] = res

    y = np.empty((NB, NS, NF), dtype=np.float32)
    for b in range(NB):
        y[b] = res.results[2 * b]["y"] + res.results[2 * b + 1]["y"] + Wproj_b
    return y
